# revision 21
# baseline (speedup 1.0000x reference)
"""Trainium2 Bass kernel for Brain3DQTUNNetwork (gnn_message_passing).

The "sparse" graph is a fixed Manhattan-radius-2 stencil on a 64^3 grid
(24 offsets).  Weights are stored dense per offset slot, keyed by the
DESTINATION (col) index: W[k][j] = w(edge j-d_k -> j), 0 for invalid
edges.  The per-step segment_sum SpMV becomes 24 shifted elementwise
multiply-accumulates, and the STDP update becomes
    w = clip(w*(1-WDECAY) + prev * q_shift,  0, 1),   q(o) = 0.015*o - 0.005.
Invalid slots self-heal to 0 every step (q reads 0 / q(0) < 0 there and
the clip floors at 0).

Layout: 128 partitions = (y-half h, x): p = h*64 + x.  Each partition
holds a 4-y-plane sub-slab: field free axis = (y_sub + halo, z + pad) =
8*68 = 544.  y/z shifts are free-dim AP offsets; x shifts are 2
per-block SBUF->SBUF DMA partition-shifted copies; the h<->h halo is an
intra-core partition+-64 DMA copy.

Sharding: 8 cores x 8 y-planes.  All state stays SBUF-resident; per-step
cross-core traffic is one 8-rank AllGather of 2-row boundary strips.

Engine use: everything elementwise on Vector (DVE); sigmoid + masked
strip copies on Scalar; GpSimd only fires the collective.  The STDP
product uses the fused affine_mul_reduce DVE op (folds q() into the
multiply).  Offset slots are grouped so stencil taps whose windows
differ by a constant stride share one instruction (24 taps -> 12 ops),
with dy==0 groups first so half the STDP/W update overlaps the
AllGather latency.
"""

import os
import sys

sys.path.insert(0, "/opt/trn_rl_repo")

import numpy as np

import bass_rust
import concourse.bass as bass
import concourse.bacc as bacc
import concourse.mybir as mybir
import concourse.tile as tile
from concourse import bass_utils

# ---- problem constants (hardcoded; kernel.py must be self-contained) ----
GRID = (64, 64, 64)
NX, NY, NZ = GRID
N = NX * NY * NZ
RADIUS = 2
NCORES = 8
YS = NY // NCORES  # y-planes per core = 8

TAU = 20.0
REST_V = -65.0
EXC_THR = -50.0
INH_THR = -70.0
RESET_V = -65.0
ETA_LTP, ETA_LTD, WDECAY = 0.01, 0.005, 1e-05

# fp32-exact scalars matching the jax reference
DECAY = float(np.exp(np.float32(-1.0 / np.float32(TAU))).astype(np.float32))
ONE_MINUS_DECAY = float(np.float32(1.0) - np.float32(DECAY))
MIDPOINT = (EXC_THR + INH_THR) / 2.0  # -60.0

# ---- offset slot grouping ----
# Each group = (dx, [(dy, dz), ...]) where consecutive taps' field windows
# differ by a constant positive stride, so one strided AP covers the whole
# group.  Part A (dy == 0, k 0..11) needs no y-halo; part B (k 12..23) does.
GROUPS_A = [
    (0, [(0, 2), (0, 1)]),            # window offsets 136,137 (stride 1)
    (0, [(0, -1), (0, -2)]),          # 139,140
    (1, [(0, 1), (0, 0), (0, -1)]),   # 137,138,139
    (-1, [(0, 1), (0, 0), (0, -1)]),
    (2, [(0, 0)]),
    (-2, [(0, 0)]),
]
GROUPS_B = [
    (0, [(2, 0)]),                    # 2
    (0, [(1, 1), (1, 0), (1, -1)]),   # 69,70,71
    (0, [(-1, 1), (-1, 0), (-1, -1)]),  # 205,206,207
    (0, [(-2, 0)]),                   # 274
    (1, [(1, 0), (-1, 0)]),           # 70,206 (stride 136)
    (-1, [(1, 0), (-1, 0)]),
]
GROUPS = GROUPS_A + GROUPS_B
OFFSETS = [(dx, dy, dz) for dx, tap in GROUPS for dy, dz in tap]
NOFF = len(OFFSETS)  # 24
NOFF_A = sum(len(tap) for _, tap in GROUPS_A)  # 12
XSHIFTS = (1, -1, 2, -2)

# field geometry: partition p = h*64 + x, free = ys*68 + zs,
# ys = y_sub + 2 in [0,8), zs = z + 2 in [0,68)
PB = 2 * NX        # 128 partitions
SUB = YS // 2      # 4 own y rows per partition block
FYS = SUB + 4      # 8 field y rows (4 own + 2 halo each side)
FZS = NZ + 4       # 68 field z cols
FFREE = FYS * FZS  # 544
CHUNK = SUB * NZ   # 256 elems per partition for compact tiles
F32 = mybir.dt.float32

_CACHE = {}


def _build_graph(nsteps):
    nc = bacc.Bacc(
        "TRN2",
        target_bir_lowering=False,
        debug=False,
        enable_asserts=True,
        num_devices=NCORES,
    )
    w0_d = nc.dram_tensor("w0", [PB, NOFF * CHUNK], F32, kind="ExternalInput").ap()
    xin_d = nc.dram_tensor("xin", [nsteps, PB, CHUNK], F32, kind="ExternalInput").ap()
    msk_d = nc.dram_tensor("msk", [PB, 1], F32, kind="ExternalInput").ap()
    spk_d = nc.dram_tensor("spk", [nsteps, PB, CHUNK], F32, kind="ExternalOutput").ap()

    AT = mybir.ActivationFunctionType
    ALU = mybir.AluOpType

    with tile.TileContext(nc) as tc, tc.tile_pool(
        name="state", bufs=1
    ) as st, tc.tile_pool(name="dram", bufs=1, space="DRAM") as dr:
        # persistent state tiles (distinct names -> distinct slots)
        W = st.tile([PB, NOFF * CHUNK], F32, name="W")
        P = st.tile([PB, NOFF * CHUNK], F32, name="P")
        FA = st.tile([PB, FFREE], F32, name="FA")
        FB = st.tile([PB, FFREE], F32, name="FB")
        SO = {dx: st.tile([PB, FFREE], F32, name=f"SOx{dx+2}") for dx in XSHIFTS}
        V = st.tile([PB, CHUNK], F32, name="V")
        SYN = st.tile([PB, CHUNK], F32, name="SYN")
        S = st.tile([PB, CHUNK], F32, name="S")
        II = st.tile([PB, CHUNK], F32, name="II")
        G = st.tile([PB, CHUNK], F32, name="G")
        NN = st.tile([PB, CHUNK], F32, name="NN")
        T1 = st.tile([PB, CHUNK], F32, name="T1")
        T2 = st.tile([PB, CHUNK], F32, name="T2")
        XIP = st.tile([PB, CHUNK], F32, name="XIP")
        MSK = st.tile([PB, 1], F32, name="MSK")
        SND = st.tile([PB, 2 * NZ], F32, name="SND")
        B30 = st.tile([PB, 1], F32, name="B30")  # sigmoid bias const
        ACC = st.tile([PB, 2 * len(GROUPS)], F32, name="ACC")  # amr scratch

        def f3(t):  # [PB, FFREE] -> [PB, FYS, FZS]
            return t.rearrange("p (y z) -> p y z", z=FZS)

        def c3(t):  # [PB, CHUNK] -> [PB, SUB, NZ]
            return t.rearrange("p (y z) -> p y z", z=NZ)

        def wg(t, k0, m):  # slots k0..k0+m of W/P -> [PB, m, SUB, NZ]
            return t.rearrange("p (k y z) -> p k y z", k=NOFF, z=NZ)[:, k0 : k0 + m]

        def fint(t):  # own interior of a field tile -> [PB, SUB, NZ]
            return f3(t)[:, 2 : 2 + SUB, 2 : 2 + NZ]

        def xshift(eng, dst, src, dx, rows):
            # dst[p, rows, :] = src[p - dx, rows, :] within each x-block
            # (one DMA per block; DMA has no partition-alignment limits)
            for h in (0, 1):
                base = h * NX
                a, b = base + max(0, dx), base + NX + min(0, dx)
                eng.dma_start(
                    f3(dst)[a:b, rows, :], f3(src)[a - dx : b - dx, rows, :]
                )

        def gwin(base, xs, g):
            # grouped source view: one AP covering every tap in group g,
            # group axis strided by the constant window-offset delta
            dx, taps = g
            t = base if dx == 0 else xs[dx]
            if len(taps) == 1:
                dy, dz = taps[0]
                return f3(t)[:, 2 - dy : 2 - dy + SUB, 2 - dz : 2 - dz + NZ]
            offs = [(2 - dy) * FZS + (2 - dz) for dy, dz in taps]
            delta = offs[1] - offs[0]
            assert delta > 0 and all(
                offs[i + 1] - offs[i] == delta for i in range(len(offs) - 1)
            )
            full = t[:]
            return bass_rust.AP(
                tensor=full.tensor,
                offset=offs[0],
                ap=[[FFREE, PB], [delta, len(taps)], [FZS, SUB], [1, NZ]],
            )

        # ---- init ----
        nc.vector.memset(FA[:], 0.0)
        nc.vector.memset(FB[:], 0.0)
        for dx in XSHIFTS:
            nc.vector.memset(SO[dx][:], 0.0)
        nc.vector.memset(V[:], REST_V)
        nc.vector.memset(B30[:], -0.5 * MIDPOINT)
        nc.sync.dma_start(W[:], w0_d[:])
        nc.sync.dma_start(MSK[:], msk_d[:])

        from concourse.ordered_set import OrderedSet

        _engs = OrderedSet([mybir.EngineType.SP, mybir.EngineType.Activation])
        pid = nc.partition_id(engines=_engs)
        # strip row indices into the gathered [8*PB, ...] buffer, in units
        # of 64 partitions: left neighbor's top strip / right's bottom strip
        selL = nc.snap(
            ((pid + NCORES - 1) % NCORES) * 2 + 1,
            engines=_engs,
            min_val=0,
            max_val=2 * NCORES - 1,
        )
        selR = nc.snap(
            ((pid + 1) % NCORES) * 2,
            engines=_engs,
            min_val=0,
            max_val=2 * NCORES - 2,
        )

        fields = [FA, FB]
        for t in range(nsteps):
            FPREV = fields[t % 2]       # holds out_{t-1} (with halos)
            FOUT = fields[(t + 1) % 2]  # will hold out_t

            if t == 0:
                # external input for step 0 (later steps prefetch at t-1)
                nc.sync.dma_start(XIP[:], xin_d[0])
                nc.scalar.mul(XIP[:], XIP[:], ONE_MINUS_DECAY)

            # ---- syn: grouped shifted products then pairwise tree sum ----
            k0 = 0
            for g in GROUPS:
                m = len(g[1])
                nc.vector.tensor_tensor(
                    wg(P, k0, m), wg(W, k0, m), gwin(FPREV, SO, g), ALU.mult
                )
                k0 += m
            # tree-reduce the 24 slots (in place over P)
            Pf = P.rearrange("p (k f) -> p k f", k=NOFF)
            nslots = NOFF
            while nslots > 1:
                half = nslots // 2
                nc.vector.tensor_tensor(
                    Pf[:, 0:half],
                    Pf[:, 0:half],
                    Pf[:, half : 2 * half],
                    ALU.add,
                )
                if nslots % 2:
                    nc.vector.tensor_tensor(
                        Pf[:, 0], Pf[:, 0], Pf[:, nslots - 1], ALU.add
                    )
                nslots = half
            # SYN = (P0 * (1-decay)) + XIP ; then v = v*decay + SYN
            nc.vector.scalar_tensor_tensor(
                SYN[:], Pf[:, 0], ONE_MINUS_DECAY, XIP[:], ALU.mult, ALU.add
            )
            nc.vector.scalar_tensor_tensor(
                V[:], V[:], DECAY, SYN[:], ALU.mult, ALU.add
            )

            # ---- neuron update (vector TS + scalar sigmoid) ----
            nc.scalar.activation(G[:], V[:], AT.Sigmoid, bias=B30[:, 0:1], scale=0.5)
            nc.vector.tensor_single_scalar(S[:], V[:], EXC_THR, ALU.is_ge)
            # spike train output for this step (early: S is final here)
            nc.sync.dma_start(spk_d[t], S[:])
            nc.vector.tensor_single_scalar(II[:], V[:], INH_THR, ALU.is_le)
            nc.vector.tensor_tensor(NN[:], S[:], II[:], ALU.add)
            nc.vector.scalar_tensor_tensor(
                T1[:], NN[:], 1.0, G[:], ALU.subtract, ALU.mult
            )
            # out = s - (n-1)*g, written straight into the FOUT interior
            nc.vector.tensor_tensor(fint(FOUT), c3(S), c3(T1), ALU.subtract)

            # ---- intra-core h<->h halo + boundary strips: launch ASAP ----
            # block 0's top halo (ys 6:8) = block 1's own rows 2:4 (p+64);
            # block 1's bottom halo (ys 0:2) = block 0's own rows 4:6 (p-64)
            nc.sync.dma_start(f3(FOUT)[0:NX, 6:8, :], f3(FOUT)[NX:PB, 2:4, :])
            nc.sync.dma_start(f3(FOUT)[NX:PB, 0:2, :], f3(FOUT)[0:NX, 4:6, :])
            # x-shift halo rows whose source is the intra copy (not the
            # network): block 0 row 6, block 1 row 1, for dx=+-1
            for dxp, eng in ((1, nc.scalar), (-1, nc.sync)):
                a, b = max(0, dxp), NX + min(0, dxp)
                eng.dma_start(
                    f3(SO[dxp])[a:b, 6:7, :], f3(FOUT)[a - dxp : b - dxp, 6:7, :]
                )
                eng.dma_start(
                    f3(SO[dxp])[NX + a : NX + b, 1:2, :],
                    f3(FOUT)[NX + a - dxp : NX + b - dxp, 1:2, :],
                )
            # strips: p<64 -> core's bottom 2 rows (ys 2:4 of block 0),
            #         p>=64 -> core's top 2 rows (ys 4:6 of block 1), masked
            snd3 = SND.rearrange("p (y z) -> p y z", z=NZ)
            nc.scalar.activation(
                snd3[0:NX], f3(FOUT)[0:NX, 2:4, 2 : 2 + NZ],
                AT.Copy, bias=0.0, scale=MSK[0:NX, 0:1],
            )
            nc.scalar.activation(
                snd3[NX:PB], f3(FOUT)[NX:PB, 4:6, 2 : 2 + NZ],
                AT.Copy, bias=0.0, scale=MSK[NX:PB, 0:1],
            )
            agin = dr.tile([PB, 2 * NZ], F32, name=f"agin{t}")
            agout = dr.tile(
                [NCORES * PB, 2 * NZ], F32, addr_space="Shared", name=f"agout{t}"
            )
            # posted from scalar, same queue as the masks -> fires immediately
            nc.scalar.dma_start(agin[:], SND[:])
            nc.gpsimd.collective_compute(
                "AllGather",
                ALU.bypass,
                replica_groups=[list(range(NCORES))],
                ins=[agin.opt()],
                outs=[agout.opt()],
            )

            # ---- off the critical path while the collective flies ----
            # urgent x-shifted copies of out_t: interior rows 2:6 only (all
            # that part A and the next step's dy==0 syn taps read), posted
            # from both DMA-capable queues in parallel
            xshift(nc.scalar, SO[1], FOUT, 1, slice(2, 6))
            xshift(nc.scalar, SO[-1], FOUT, -1, slice(2, 6))
            xshift(nc.sync, SO[2], FOUT, 2, slice(2, 6))
            xshift(nc.sync, SO[-2], FOUT, -2, slice(2, 6))
            # prefetch + prescale next step's external input (WAR on this
            # step's SYN read is tracked automatically)
            if t + 1 < nsteps:
                nc.sync.dma_start(XIP[:], xin_d[t + 1])
                nc.scalar.mul(XIP[:], XIP[:], ONE_MINUS_DECAY)
            # v reset where spiked: v -= (v - RESET_V) * s
            nc.vector.scalar_tensor_tensor(
                T2[:], V[:], -RESET_V, S[:], ALU.add, ALU.mult
            )
            nc.vector.tensor_tensor(V[:], V[:], T2[:], ALU.subtract)

            # ---- STDP part A: dy == 0 groups need no y-halo (t=0 skips) ----
            # dw[k][j] = prev[j] * (0.015*out[j-dk] - 0.005) via fused DVE op
            if t > 0:
                for k, d in enumerate(OFFSETS[:NOFF_A]):
                    dxk, dy, dz = d
                    src = FOUT if dxk == 0 else SO[dxk]
                    nc.vector.affine_mul_reduce(
                        wg(P, k, 1).squeeze(1),
                        ACC[:, k : k + 1],
                        f3(src)[:, 2 - dy : 2 - dy + SUB, 2 - dz : 2 - dz + NZ],
                        fint(FPREV),
                        ETA_LTP + ETA_LTD,
                        -ETA_LTD,
                    )
                nc.vector.scalar_tensor_tensor(
                    W[:, 0 : NOFF_A * CHUNK],
                    W[:, 0 : NOFF_A * CHUNK],
                    1.0 - WDECAY,
                    P[:, 0 : NOFF_A * CHUNK],
                    ALU.mult,
                    ALU.add,
                )
                nc.vector.tensor_scalar(
                    W[:, 0 : NOFF_A * CHUNK],
                    W[:, 0 : NOFF_A * CHUNK],
                    1.0,
                    0.0,
                    ALU.min,
                    ALU.max,
                )

            # ---- halo extraction (gated on the collective; posts spread
            # across sync+scalar queues so they all fire in parallel) ----
            agf = agout.rearrange("p (y z) -> p y z", z=NZ)
            # left neighbor's top strip -> block 0's bottom halo (ys 0:2)
            nc.sync.dma_start(
                f3(FOUT)[0:NX, 0:2, 2 : 2 + NZ],
                agf[bass.ds(selL * NX, NX)],
            )
            # right neighbor's bottom strip -> block 1's top halo (ys 6:8)
            nc.scalar.dma_start(
                f3(FOUT)[NX:PB, 6:8, 2 : 2 + NZ],
                agf[bass.ds(selR * NX, NX)],
            )
            # x-shifted copies of the exchanged halo rows, read straight out
            # of the gathered buffer with a partition offset (no extract ->
            # patch chain): SO[dx] block 0 row 1 and block 1 row 6
            for dxp, engs in ((1, (nc.sync, nc.scalar)), (-1, (nc.scalar, nc.sync))):
                a, b = max(0, dxp), NX + min(0, dxp)
                engs[0].dma_start(
                    f3(SO[dxp])[a:b, 1:2, 2 : 2 + NZ],
                    agf[bass.ds(selL * NX + a - dxp, b - a), 1:2],
                )
                engs[1].dma_start(
                    f3(SO[dxp])[NX + a : NX + b, 6:7, 2 : 2 + NZ],
                    agf[bass.ds(selR * NX + a - dxp, b - a), 0:1],
                )

            # ---- STDP part B: dy != 0 groups (need the fresh halo) ----
            if t > 0:
                for k, d in enumerate(OFFSETS[NOFF_A:], start=NOFF_A):
                    dxk, dy, dz = d
                    src = FOUT if dxk == 0 else SO[dxk]
                    nc.vector.affine_mul_reduce(
                        wg(P, k, 1).squeeze(1),
                        ACC[:, k : k + 1],
                        f3(src)[:, 2 - dy : 2 - dy + SUB, 2 - dz : 2 - dz + NZ],
                        fint(FPREV),
                        ETA_LTP + ETA_LTD,
                        -ETA_LTD,
                    )
                nc.vector.scalar_tensor_tensor(
                    W[:, NOFF_A * CHUNK :],
                    W[:, NOFF_A * CHUNK :],
                    1.0 - WDECAY,
                    P[:, NOFF_A * CHUNK :],
                    ALU.mult,
                    ALU.add,
                )
                nc.vector.tensor_scalar(
                    W[:, NOFF_A * CHUNK :],
                    W[:, NOFF_A * CHUNK :],
                    1.0,
                    0.0,
                    ALU.min,
                    ALU.max,
                )

    nc.compile()
    return nc


def _shard_inputs(external_input, edge_values, edge_rows, edge_cols, nsteps):
    """Build per-core input maps (host-side sharding)."""
    ext = np.ascontiguousarray(np.asarray(external_input, dtype=np.float32))[:nsteps]
    vals = np.asarray(edge_values, dtype=np.float32)
    rows = np.asarray(edge_rows, dtype=np.int64)
    cols = np.asarray(edge_cols, dtype=np.int64)

    # dense weights keyed by destination: Wd[k, j] = w(edge j-d_k -> j)
    dlin = cols - rows
    offs_lin = np.array([d[0] * NY * NZ + d[1] * NZ + d[2] for d in OFFSETS])
    assert set(int(v) for v in np.unique(dlin)).issubset(
        set(int(v) for v in offs_lin)
    )
    k_of = np.zeros(int(offs_lin.max()) - int(offs_lin.min()) + 1, dtype=np.int64)
    for i, v in enumerate(offs_lin):
        k_of[int(v) - int(offs_lin.min())] = i
    ke = k_of[dlin - int(offs_lin.min())]
    Wd = np.zeros((NOFF, N), dtype=np.float32)
    Wd[ke, cols] = vals

    # [NOFF, NX, NCORES, 2(h), SUB, NZ]
    Wd = Wd.reshape(NOFF, NX, NCORES, 2, SUB, NZ)
    ext = ext.reshape(nsteps, NX, NCORES, 2, SUB, NZ)

    in_maps = []
    for c in range(NCORES):
        # partition p = h*64 + x
        wc = np.ascontiguousarray(
            Wd[:, :, c].transpose(2, 1, 0, 3, 4)
        ).reshape(PB, NOFF * CHUNK)
        xc = np.ascontiguousarray(
            ext[:, :, c].transpose(0, 2, 1, 3, 4)
        ).reshape(nsteps, PB, CHUNK)
        msk = np.zeros((PB, 1), dtype=np.float32)
        msk[0:NX, 0] = 0.0 if c == 0 else 1.0           # bottom strip valid?
        msk[NX:PB, 0] = 0.0 if c == NCORES - 1 else 1.0  # top strip valid?
        in_maps.append({"w0": wc, "xin": xc, "msk": msk})
    return in_maps


def kernel(external_input, edge_values, edge_rows, edge_cols, num_steps):
    nsteps = int(num_steps)
    if nsteps not in _CACHE:
        _CACHE[nsteps] = _build_graph(nsteps)
    nc = _CACHE[nsteps]

    in_maps = _shard_inputs(external_input, edge_values, edge_rows, edge_cols, nsteps)
    res = bass_utils.run_bass_kernel_spmd(
        nc,
        in_maps,
        core_ids=list(range(NCORES)),
        trace=bool(int(os.environ.get("BRAIN_TRACE", "0"))),
    )

    out = np.empty((nsteps, NX, NCORES, 2, SUB, NZ), dtype=np.float32)
    for c in range(NCORES):
        out[:, :, c] = (
            res.results[c]["spk"]
            .reshape(nsteps, 2, NX, SUB, NZ)
            .transpose(0, 2, 1, 3, 4)
        )
    kernel.last_results = res
    return out.reshape(nsteps, N)


# revision 23
# speedup vs baseline: 1.0679x; 1.0679x over previous
"""Trainium2 Bass kernel for Brain3DQTUNNetwork (gnn_message_passing).

The "sparse" graph is a fixed Manhattan-radius-2 stencil on a 64^3 grid
(24 offsets).  Weights are stored dense per offset slot, keyed by the
DESTINATION (col) index: W[k][j] = w(edge j-d_k -> j), 0 for invalid
edges.  The per-step segment_sum SpMV becomes 24 shifted elementwise
multiply-accumulates, and the STDP update becomes
    w = clip(w*(1-WDECAY) + prev * q_shift,  0, 1),   q(o) = 0.015*o - 0.005.
Invalid slots self-heal to 0 every step (q reads 0 / q(0) < 0 there and
the clip floors at 0).

Layout: 128 partitions = (y-half h, x): p = h*64 + x.  Each partition
holds a 4-y-plane sub-slab: field free axis = (y_sub + halo, z + pad) =
8*68 = 544.  y/z shifts are free-dim AP offsets; x shifts are 2
per-block SBUF->SBUF DMA partition-shifted copies; the h<->h halo is an
intra-core partition+-64 DMA copy.

Sharding: 8 cores x 8 y-planes.  All state stays SBUF-resident; per-step
cross-core traffic is one 8-rank AllGather of 2-row boundary strips.

Engine use: everything elementwise on Vector (DVE); sigmoid + masked
strip copies on Scalar; GpSimd only fires the collective.  The STDP
product uses the fused affine_mul_reduce DVE op (folds q() into the
multiply).  Offset slots are grouped so stencil taps whose windows
differ by a constant stride share one instruction (24 taps -> 12 ops),
with dy==0 groups first so half the STDP/W update overlaps the
AllGather latency.
"""

import os
import sys

sys.path.insert(0, "/opt/trn_rl_repo")

import numpy as np

import bass_rust
import concourse.bass as bass
import concourse.bacc as bacc
import concourse.mybir as mybir
import concourse.tile as tile
from concourse import bass_utils

# ---- problem constants (hardcoded; kernel.py must be self-contained) ----
GRID = (64, 64, 64)
NX, NY, NZ = GRID
N = NX * NY * NZ
RADIUS = 2
NCORES = 8
YS = NY // NCORES  # y-planes per core = 8

TAU = 20.0
REST_V = -65.0
EXC_THR = -50.0
INH_THR = -70.0
RESET_V = -65.0
ETA_LTP, ETA_LTD, WDECAY = 0.01, 0.005, 1e-05

# fp32-exact scalars matching the jax reference
DECAY = float(np.exp(np.float32(-1.0 / np.float32(TAU))).astype(np.float32))
ONE_MINUS_DECAY = float(np.float32(1.0) - np.float32(DECAY))
MIDPOINT = (EXC_THR + INH_THR) / 2.0  # -60.0

# ---- offset slot grouping ----
# Each group = (dx, [(dy, dz), ...]) where consecutive taps' field windows
# differ by a constant positive stride, so one strided AP covers the whole
# group.  Part A (dy == 0, k 0..11) needs no y-halo; part B (k 12..23) does.
GROUPS_A = [
    (0, [(0, 2), (0, 1)]),            # window offsets 136,137 (stride 1)
    (0, [(0, -1), (0, -2)]),          # 139,140
    (1, [(0, 1), (0, 0), (0, -1)]),   # 137,138,139
    (-1, [(0, 1), (0, 0), (0, -1)]),
    (2, [(0, 0)]),
    (-2, [(0, 0)]),
]
GROUPS_B = [
    (0, [(2, 0)]),                    # 2
    (0, [(1, 1), (1, 0), (1, -1)]),   # 69,70,71
    (0, [(-1, 1), (-1, 0), (-1, -1)]),  # 205,206,207
    (0, [(-2, 0)]),                   # 274
    (1, [(1, 0), (-1, 0)]),           # 70,206 (stride 136)
    (-1, [(1, 0), (-1, 0)]),
]
GROUPS = GROUPS_A + GROUPS_B
OFFSETS = [(dx, dy, dz) for dx, tap in GROUPS for dy, dz in tap]
NOFF = len(OFFSETS)  # 24
NOFF_A = sum(len(tap) for _, tap in GROUPS_A)  # 12
XSHIFTS = (1, -1, 2, -2)

# field geometry: partition p = h*64 + x, free = ys*68 + zs,
# ys = y_sub + 2 in [0,8), zs = z + 2 in [0,68)
PB = 2 * NX        # 128 partitions
SUB = YS // 2      # 4 own y rows per partition block
FYS = SUB + 4      # 8 field y rows (4 own + 2 halo each side)
FZS = NZ + 4       # 68 field z cols
FFREE = FYS * FZS  # 544
CHUNK = SUB * NZ   # 256 elems per partition for compact tiles
F32 = mybir.dt.float32

_CACHE = {}


def _build_graph(nsteps):
    nc = bacc.Bacc(
        "TRN2",
        target_bir_lowering=False,
        debug=False,
        enable_asserts=True,
        num_devices=NCORES,
    )
    w0_d = nc.dram_tensor("w0", [PB, NOFF * CHUNK], F32, kind="ExternalInput").ap()
    xin_d = nc.dram_tensor("xin", [nsteps, PB, CHUNK], F32, kind="ExternalInput").ap()
    msk_d = nc.dram_tensor("msk", [PB, 1], F32, kind="ExternalInput").ap()
    spk_d = nc.dram_tensor("spk", [nsteps, PB, CHUNK], F32, kind="ExternalOutput").ap()

    AT = mybir.ActivationFunctionType
    ALU = mybir.AluOpType

    with tile.TileContext(nc) as tc, tc.tile_pool(
        name="state", bufs=1
    ) as st, tc.tile_pool(name="dram", bufs=1, space="DRAM") as dr:
        # persistent state tiles (distinct names -> distinct slots)
        W = st.tile([PB, NOFF * CHUNK], F32, name="W")
        P = st.tile([PB, NOFF * CHUNK], F32, name="P")
        PH = st.tile([PB, NOFF * CHUNK], mybir.dt.float16, name="PH")
        FA = st.tile([PB, FFREE], F32, name="FA")
        FB = st.tile([PB, FFREE], F32, name="FB")
        SO = {dx: st.tile([PB, FFREE], F32, name=f"SOx{dx+2}") for dx in XSHIFTS}
        V = st.tile([PB, CHUNK], F32, name="V")
        SYN = st.tile([PB, CHUNK], F32, name="SYN")
        S = st.tile([PB, CHUNK], F32, name="S")
        II = st.tile([PB, CHUNK], F32, name="II")
        G = st.tile([PB, CHUNK], F32, name="G")
        NN = st.tile([PB, CHUNK], F32, name="NN")
        T1 = st.tile([PB, CHUNK], F32, name="T1")
        T2 = st.tile([PB, CHUNK], F32, name="T2")
        XIP = st.tile([PB, CHUNK], F32, name="XIP")
        MSK = st.tile([PB, 1], F32, name="MSK")
        SND = st.tile([PB, 2 * NZ], F32, name="SND")
        B30 = st.tile([PB, 1], F32, name="B30")  # sigmoid bias const
        ACC = st.tile([PB, 2 * len(GROUPS)], F32, name="ACC")  # amr scratch

        def f3(t):  # [PB, FFREE] -> [PB, FYS, FZS]
            return t.rearrange("p (y z) -> p y z", z=FZS)

        def c3(t):  # [PB, CHUNK] -> [PB, SUB, NZ]
            return t.rearrange("p (y z) -> p y z", z=NZ)

        def wg(t, k0, m):  # slots k0..k0+m of W/P -> [PB, m, SUB, NZ]
            return t.rearrange("p (k y z) -> p k y z", k=NOFF, z=NZ)[:, k0 : k0 + m]

        def fint(t):  # own interior of a field tile -> [PB, SUB, NZ]
            return f3(t)[:, 2 : 2 + SUB, 2 : 2 + NZ]

        def xshift(eng, dst, src, dx, rows):
            # dst[p, rows, :] = src[p - dx, rows, :] within each x-block
            # (one DMA per block; DMA has no partition-alignment limits)
            for h in (0, 1):
                base = h * NX
                a, b = base + max(0, dx), base + NX + min(0, dx)
                eng.dma_start(
                    f3(dst)[a:b, rows, :], f3(src)[a - dx : b - dx, rows, :]
                )

        def gwin(base, xs, g):
            # grouped source view: one AP covering every tap in group g,
            # group axis strided by the constant window-offset delta
            dx, taps = g
            t = base if dx == 0 else xs[dx]
            if len(taps) == 1:
                dy, dz = taps[0]
                return f3(t)[:, 2 - dy : 2 - dy + SUB, 2 - dz : 2 - dz + NZ]
            offs = [(2 - dy) * FZS + (2 - dz) for dy, dz in taps]
            delta = offs[1] - offs[0]
            assert delta > 0 and all(
                offs[i + 1] - offs[i] == delta for i in range(len(offs) - 1)
            )
            full = t[:]
            return bass_rust.AP(
                tensor=full.tensor,
                offset=offs[0],
                ap=[[FFREE, PB], [delta, len(taps)], [FZS, SUB], [1, NZ]],
            )

        # ---- init ----
        nc.vector.memset(FA[:], 0.0)
        nc.vector.memset(FB[:], 0.0)
        for dx in XSHIFTS:
            nc.vector.memset(SO[dx][:], 0.0)
        nc.vector.memset(V[:], REST_V)
        nc.vector.memset(B30[:], -0.5 * MIDPOINT)
        nc.sync.dma_start(W[:], w0_d[:])
        nc.sync.dma_start(MSK[:], msk_d[:])

        from concourse.ordered_set import OrderedSet

        _engs = OrderedSet([mybir.EngineType.SP, mybir.EngineType.Activation])
        pid = nc.partition_id(engines=_engs)
        # strip row indices into the gathered [8*PB, ...] buffer, in units
        # of 64 partitions: left neighbor's top strip / right's bottom strip
        selL = nc.snap(
            ((pid + NCORES - 1) % NCORES) * 2 + 1,
            engines=_engs,
            min_val=0,
            max_val=2 * NCORES - 1,
        )
        selR = nc.snap(
            ((pid + 1) % NCORES) * 2,
            engines=_engs,
            min_val=0,
            max_val=2 * NCORES - 2,
        )

        fields = [FA, FB]
        for t in range(nsteps):
            FPREV = fields[t % 2]       # holds out_{t-1} (with halos)
            FOUT = fields[(t + 1) % 2]  # will hold out_t

            if t == 0:
                # external input for step 0 (later steps prefetch at t-1)
                nc.sync.dma_start(XIP[:], xin_d[0])
                nc.scalar.mul(XIP[:], XIP[:], ONE_MINUS_DECAY)

            # ---- syn: grouped shifted products then pairwise tree sum ----
            # products + tree in fp16 (packed 16-bit runs the DVE adds at 2
            # elem/cycle; spike outputs are binary so the ~1e-3 relative
            # rounding only matters within that distance of the threshold)
            k0 = 0
            for g in GROUPS:
                m = len(g[1])
                nc.vector.tensor_tensor(
                    wg(PH, k0, m), wg(W, k0, m), gwin(FPREV, SO, g), ALU.mult
                )
                k0 += m
            # tree-reduce the 24 slots (in place over PH)
            Pf = PH.rearrange("p (k f) -> p k f", k=NOFF)
            nslots = NOFF
            while nslots > 1:
                half = nslots // 2
                nc.vector.tensor_tensor(
                    Pf[:, 0:half],
                    Pf[:, 0:half],
                    Pf[:, half : 2 * half],
                    ALU.add,
                )
                if nslots % 2:
                    nc.vector.tensor_tensor(
                        Pf[:, 0], Pf[:, 0], Pf[:, nslots - 1], ALU.add
                    )
                nslots = half
            # SYN = (P0 * (1-decay)) + XIP ; then v = v*decay + SYN
            nc.vector.scalar_tensor_tensor(
                SYN[:], Pf[:, 0], ONE_MINUS_DECAY, XIP[:], ALU.mult, ALU.add
            )
            nc.vector.scalar_tensor_tensor(
                V[:], V[:], DECAY, SYN[:], ALU.mult, ALU.add
            )

            # ---- neuron update (vector TS + scalar sigmoid) ----
            nc.scalar.activation(G[:], V[:], AT.Sigmoid, bias=B30[:, 0:1], scale=0.5)
            nc.vector.tensor_single_scalar(S[:], V[:], EXC_THR, ALU.is_ge)
            # spike train output for this step (early: S is final here)
            nc.sync.dma_start(spk_d[t], S[:])
            nc.vector.tensor_single_scalar(II[:], V[:], INH_THR, ALU.is_le)
            nc.vector.tensor_tensor(NN[:], S[:], II[:], ALU.add)
            nc.vector.scalar_tensor_tensor(
                T1[:], NN[:], 1.0, G[:], ALU.subtract, ALU.mult
            )
            # out = s - (n-1)*g, written straight into the FOUT interior
            nc.vector.tensor_tensor(fint(FOUT), c3(S), c3(T1), ALU.subtract)

            # ---- intra-core h<->h halo + boundary strips: launch ASAP ----
            # block 0's top halo (ys 6:8) = block 1's own rows 2:4 (p+64);
            # block 1's bottom halo (ys 0:2) = block 0's own rows 4:6 (p-64)
            nc.sync.dma_start(f3(FOUT)[0:NX, 6:8, :], f3(FOUT)[NX:PB, 2:4, :])
            nc.sync.dma_start(f3(FOUT)[NX:PB, 0:2, :], f3(FOUT)[0:NX, 4:6, :])
            # x-shift halo rows whose source is the intra copy (not the
            # network): block 0 row 6, block 1 row 1, for dx=+-1
            for dxp, eng in ((1, nc.scalar), (-1, nc.sync)):
                a, b = max(0, dxp), NX + min(0, dxp)
                eng.dma_start(
                    f3(SO[dxp])[a:b, 6:7, :], f3(FOUT)[a - dxp : b - dxp, 6:7, :]
                )
                eng.dma_start(
                    f3(SO[dxp])[NX + a : NX + b, 1:2, :],
                    f3(FOUT)[NX + a - dxp : NX + b - dxp, 1:2, :],
                )
            # strips: p<64 -> core's bottom 2 rows (ys 2:4 of block 0),
            #         p>=64 -> core's top 2 rows (ys 4:6 of block 1), masked
            snd3 = SND.rearrange("p (y z) -> p y z", z=NZ)
            nc.scalar.activation(
                snd3[0:NX], f3(FOUT)[0:NX, 2:4, 2 : 2 + NZ],
                AT.Copy, bias=0.0, scale=MSK[0:NX, 0:1],
            )
            nc.scalar.activation(
                snd3[NX:PB], f3(FOUT)[NX:PB, 4:6, 2 : 2 + NZ],
                AT.Copy, bias=0.0, scale=MSK[NX:PB, 0:1],
            )
            agin = dr.tile([PB, 2 * NZ], F32, name=f"agin{t}")
            agout = dr.tile(
                [NCORES * PB, 2 * NZ], F32, addr_space="Shared", name=f"agout{t}"
            )
            # posted from scalar, same queue as the masks -> fires immediately
            nc.scalar.dma_start(agin[:], SND[:])
            nc.gpsimd.collective_compute(
                "AllGather",
                ALU.bypass,
                replica_groups=[list(range(NCORES))],
                ins=[agin.opt()],
                outs=[agout.opt()],
            )

            # ---- off the critical path while the collective flies ----
            # urgent x-shifted copies of out_t: interior rows 2:6 only (all
            # that part A and the next step's dy==0 syn taps read), posted
            # from both DMA-capable queues in parallel
            xshift(nc.scalar, SO[1], FOUT, 1, slice(2, 6))
            xshift(nc.scalar, SO[-1], FOUT, -1, slice(2, 6))
            xshift(nc.sync, SO[2], FOUT, 2, slice(2, 6))
            xshift(nc.sync, SO[-2], FOUT, -2, slice(2, 6))
            # prefetch + prescale next step's external input (WAR on this
            # step's SYN read is tracked automatically)
            if t + 1 < nsteps:
                nc.sync.dma_start(XIP[:], xin_d[t + 1])
                nc.scalar.mul(XIP[:], XIP[:], ONE_MINUS_DECAY)
            # v reset where spiked: v -= (v - RESET_V) * s
            nc.vector.scalar_tensor_tensor(
                T2[:], V[:], -RESET_V, S[:], ALU.add, ALU.mult
            )
            nc.vector.tensor_tensor(V[:], V[:], T2[:], ALU.subtract)

            # ---- STDP part A: dy == 0 groups need no y-halo (t=0 skips) ----
            # dw[k][j] = prev[j] * (0.015*out[j-dk] - 0.005) via fused DVE op
            if t > 0:
                for k, d in enumerate(OFFSETS[:NOFF_A]):
                    dxk, dy, dz = d
                    src = FOUT if dxk == 0 else SO[dxk]
                    nc.vector.affine_mul_reduce(
                        wg(P, k, 1).squeeze(1),
                        ACC[:, k : k + 1],
                        f3(src)[:, 2 - dy : 2 - dy + SUB, 2 - dz : 2 - dz + NZ],
                        fint(FPREV),
                        ETA_LTP + ETA_LTD,
                        -ETA_LTD,
                    )
                nc.vector.scalar_tensor_tensor(
                    W[:, 0 : NOFF_A * CHUNK],
                    W[:, 0 : NOFF_A * CHUNK],
                    1.0 - WDECAY,
                    P[:, 0 : NOFF_A * CHUNK],
                    ALU.mult,
                    ALU.add,
                )
                nc.vector.tensor_scalar(
                    W[:, 0 : NOFF_A * CHUNK],
                    W[:, 0 : NOFF_A * CHUNK],
                    1.0,
                    0.0,
                    ALU.min,
                    ALU.max,
                )

            # ---- halo extraction (gated on the collective; posts spread
            # across sync+scalar queues so they all fire in parallel) ----
            agf = agout.rearrange("p (y z) -> p y z", z=NZ)
            # left neighbor's top strip -> block 0's bottom halo (ys 0:2)
            nc.sync.dma_start(
                f3(FOUT)[0:NX, 0:2, 2 : 2 + NZ],
                agf[bass.ds(selL * NX, NX)],
            )
            # right neighbor's bottom strip -> block 1's top halo (ys 6:8)
            nc.scalar.dma_start(
                f3(FOUT)[NX:PB, 6:8, 2 : 2 + NZ],
                agf[bass.ds(selR * NX, NX)],
            )
            # x-shifted copies of the exchanged halo rows, read straight out
            # of the gathered buffer with a partition offset (no extract ->
            # patch chain): SO[dx] block 0 row 1 and block 1 row 6
            for dxp, engs in ((1, (nc.sync, nc.scalar)), (-1, (nc.scalar, nc.sync))):
                a, b = max(0, dxp), NX + min(0, dxp)
                engs[0].dma_start(
                    f3(SO[dxp])[a:b, 1:2, 2 : 2 + NZ],
                    agf[bass.ds(selL * NX + a - dxp, b - a), 1:2],
                )
                engs[1].dma_start(
                    f3(SO[dxp])[NX + a : NX + b, 6:7, 2 : 2 + NZ],
                    agf[bass.ds(selR * NX + a - dxp, b - a), 0:1],
                )

            # ---- STDP part B: dy != 0 groups (need the fresh halo) ----
            if t > 0:
                for k, d in enumerate(OFFSETS[NOFF_A:], start=NOFF_A):
                    dxk, dy, dz = d
                    src = FOUT if dxk == 0 else SO[dxk]
                    nc.vector.affine_mul_reduce(
                        wg(P, k, 1).squeeze(1),
                        ACC[:, k : k + 1],
                        f3(src)[:, 2 - dy : 2 - dy + SUB, 2 - dz : 2 - dz + NZ],
                        fint(FPREV),
                        ETA_LTP + ETA_LTD,
                        -ETA_LTD,
                    )
                nc.vector.scalar_tensor_tensor(
                    W[:, NOFF_A * CHUNK :],
                    W[:, NOFF_A * CHUNK :],
                    1.0 - WDECAY,
                    P[:, NOFF_A * CHUNK :],
                    ALU.mult,
                    ALU.add,
                )
                nc.vector.tensor_scalar(
                    W[:, NOFF_A * CHUNK :],
                    W[:, NOFF_A * CHUNK :],
                    1.0,
                    0.0,
                    ALU.min,
                    ALU.max,
                )

    nc.compile()
    return nc


def _shard_inputs(external_input, edge_values, edge_rows, edge_cols, nsteps):
    """Build per-core input maps (host-side sharding)."""
    ext = np.ascontiguousarray(np.asarray(external_input, dtype=np.float32))[:nsteps]
    vals = np.asarray(edge_values, dtype=np.float32)
    rows = np.asarray(edge_rows, dtype=np.int64)
    cols = np.asarray(edge_cols, dtype=np.int64)

    # dense weights keyed by destination: Wd[k, j] = w(edge j-d_k -> j)
    dlin = cols - rows
    offs_lin = np.array([d[0] * NY * NZ + d[1] * NZ + d[2] for d in OFFSETS])
    assert set(int(v) for v in np.unique(dlin)).issubset(
        set(int(v) for v in offs_lin)
    )
    k_of = np.zeros(int(offs_lin.max()) - int(offs_lin.min()) + 1, dtype=np.int64)
    for i, v in enumerate(offs_lin):
        k_of[int(v) - int(offs_lin.min())] = i
    ke = k_of[dlin - int(offs_lin.min())]
    Wd = np.zeros((NOFF, N), dtype=np.float32)
    Wd[ke, cols] = vals

    # [NOFF, NX, NCORES, 2(h), SUB, NZ]
    Wd = Wd.reshape(NOFF, NX, NCORES, 2, SUB, NZ)
    ext = ext.reshape(nsteps, NX, NCORES, 2, SUB, NZ)

    in_maps = []
    for c in range(NCORES):
        # partition p = h*64 + x
        wc = np.ascontiguousarray(
            Wd[:, :, c].transpose(2, 1, 0, 3, 4)
        ).reshape(PB, NOFF * CHUNK)
        xc = np.ascontiguousarray(
            ext[:, :, c].transpose(0, 2, 1, 3, 4)
        ).reshape(nsteps, PB, CHUNK)
        msk = np.zeros((PB, 1), dtype=np.float32)
        msk[0:NX, 0] = 0.0 if c == 0 else 1.0           # bottom strip valid?
        msk[NX:PB, 0] = 0.0 if c == NCORES - 1 else 1.0  # top strip valid?
        in_maps.append({"w0": wc, "xin": xc, "msk": msk})
    return in_maps


def kernel(external_input, edge_values, edge_rows, edge_cols, num_steps):
    nsteps = int(num_steps)
    if nsteps not in _CACHE:
        _CACHE[nsteps] = _build_graph(nsteps)
    nc = _CACHE[nsteps]

    in_maps = _shard_inputs(external_input, edge_values, edge_rows, edge_cols, nsteps)
    res = bass_utils.run_bass_kernel_spmd(
        nc,
        in_maps,
        core_ids=list(range(NCORES)),
        trace=bool(int(os.environ.get("BRAIN_TRACE", "0"))),
    )

    out = np.empty((nsteps, NX, NCORES, 2, SUB, NZ), dtype=np.float32)
    for c in range(NCORES):
        out[:, :, c] = (
            res.results[c]["spk"]
            .reshape(nsteps, 2, NX, SUB, NZ)
            .transpose(0, 2, 1, 3, 4)
        )
    kernel.last_results = res
    return out.reshape(nsteps, N)


# revision 29
# speedup vs baseline: 1.0740x; 1.0057x over previous
"""Trainium2 Bass kernel for Brain3DQTUNNetwork (gnn_message_passing).

The "sparse" graph is a fixed Manhattan-radius-2 stencil on a 64^3 grid
(24 offsets).  Weights are stored dense per offset slot, keyed by the
DESTINATION (col) index: W[k][j] = w(edge j-d_k -> j), 0 for invalid
edges.  The per-step segment_sum SpMV becomes 24 shifted elementwise
multiply-accumulates, and the STDP update becomes
    w = clip(w*(1-WDECAY) + prev * q_shift,  0, 1),   q(o) = 0.015*o - 0.005.
Invalid slots self-heal to 0 every step (q reads 0 / q(0) < 0 there and
the clip floors at 0).

Layout: 128 partitions = (y-half h, x): p = h*64 + x.  Each partition
holds a 4-y-plane sub-slab: field free axis = (y_sub + halo, z + pad) =
8*68 = 544.  y/z shifts are free-dim AP offsets; x shifts are 2
per-block SBUF->SBUF DMA partition-shifted copies; the h<->h halo is an
intra-core partition+-64 DMA copy.

Sharding: 8 cores x 8 y-planes.  All state stays SBUF-resident; per-step
cross-core traffic is one 8-rank AllGather of 2-row boundary strips.

Engine use: everything elementwise on Vector (DVE); sigmoid + masked
strip copies on Scalar; GpSimd only fires the collective.  The STDP
product uses the fused affine_mul_reduce DVE op (folds q() into the
multiply).  Offset slots are grouped so stencil taps whose windows
differ by a constant stride share one instruction (24 taps -> 12 ops),
with dy==0 groups first so half the STDP/W update overlaps the
AllGather latency.
"""

import os
import sys

sys.path.insert(0, "/opt/trn_rl_repo")

import numpy as np

import bass_rust
import concourse.bass as bass
import concourse.bacc as bacc
import concourse.mybir as mybir
import concourse.tile as tile
from concourse import bass_utils

# ---- problem constants (hardcoded; kernel.py must be self-contained) ----
GRID = (64, 64, 64)
NX, NY, NZ = GRID
N = NX * NY * NZ
RADIUS = 2
NCORES = 8
YS = NY // NCORES  # y-planes per core = 8

TAU = 20.0
REST_V = -65.0
EXC_THR = -50.0
INH_THR = -70.0
RESET_V = -65.0
ETA_LTP, ETA_LTD, WDECAY = 0.01, 0.005, 1e-05

# fp32-exact scalars matching the jax reference
DECAY = float(np.exp(np.float32(-1.0 / np.float32(TAU))).astype(np.float32))
ONE_MINUS_DECAY = float(np.float32(1.0) - np.float32(DECAY))
MIDPOINT = (EXC_THR + INH_THR) / 2.0  # -60.0

# ---- offset slot grouping ----
# Each group = (dx, [(dy, dz), ...]) where consecutive taps' field windows
# differ by a constant positive stride, so one strided AP covers the whole
# group.  Part A (dy == 0, k 0..11) needs no y-halo; part B (k 12..23) does.
GROUPS_A = [
    (0, [(0, 2), (0, 1)]),            # window offsets 136,137 (stride 1)
    (0, [(0, -1), (0, -2)]),          # 139,140
    (1, [(0, 1), (0, 0), (0, -1)]),   # 137,138,139
    (-1, [(0, 1), (0, 0), (0, -1)]),
    (2, [(0, 0)]),
    (-2, [(0, 0)]),
]
GROUPS_B = [
    (0, [(2, 0)]),                    # 2
    (0, [(1, 1), (1, 0), (1, -1)]),   # 69,70,71
    (0, [(-1, 1), (-1, 0), (-1, -1)]),  # 205,206,207
    (0, [(-2, 0)]),                   # 274
    (1, [(1, 0), (-1, 0)]),           # 70,206 (stride 136)
    (-1, [(1, 0), (-1, 0)]),
]
GROUPS = GROUPS_A + GROUPS_B
OFFSETS = [(dx, dy, dz) for dx, tap in GROUPS for dy, dz in tap]
NOFF = len(OFFSETS)  # 24
NOFF_A = sum(len(tap) for _, tap in GROUPS_A)  # 12
XSHIFTS = (1, -1, 2, -2)

# field geometry: partition p = h*64 + x, free = ys*68 + zs,
# ys = y_sub + 2 in [0,8), zs = z + 2 in [0,68)
PB = 2 * NX        # 128 partitions
SUB = YS // 2      # 4 own y rows per partition block
FYS = SUB + 4      # 8 field y rows (4 own + 2 halo each side)
FZS = NZ + 4       # 68 field z cols
FFREE = FYS * FZS  # 544
CHUNK = SUB * NZ   # 256 elems per partition for compact tiles
F32 = mybir.dt.float32

_CACHE = {}


def _build_graph(nsteps):
    nc = bacc.Bacc(
        "TRN2",
        target_bir_lowering=False,
        debug=False,
        enable_asserts=True,
        num_devices=NCORES,
    )
    w0_d = nc.dram_tensor("w0", [PB, NOFF * CHUNK], F32, kind="ExternalInput").ap()
    xin_d = nc.dram_tensor("xin", [nsteps, PB, CHUNK], F32, kind="ExternalInput").ap()
    msk_d = nc.dram_tensor("msk", [PB, 1], F32, kind="ExternalInput").ap()
    spk_d = nc.dram_tensor("spk", [nsteps, PB, CHUNK], F32, kind="ExternalOutput").ap()

    AT = mybir.ActivationFunctionType
    ALU = mybir.AluOpType

    with tile.TileContext(nc) as tc, tc.tile_pool(
        name="state", bufs=1
    ) as st, tc.tile_pool(name="dram", bufs=1, space="DRAM") as dr:
        # persistent state tiles (distinct names -> distinct slots)
        W = st.tile([PB, NOFF * CHUNK], F32, name="W")
        P = st.tile([PB, NOFF * CHUNK], F32, name="P")
        PH = st.tile([PB, NOFF * CHUNK], mybir.dt.float16, name="PH")
        FA = st.tile([PB, FFREE], F32, name="FA")
        FB = st.tile([PB, FFREE], F32, name="FB")
        SO = {dx: st.tile([PB, FFREE], F32, name=f"SOx{dx+2}") for dx in XSHIFTS}
        V = st.tile([PB, CHUNK], F32, name="V")
        SYN = st.tile([PB, CHUNK], F32, name="SYN")
        S = st.tile([PB, CHUNK], F32, name="S")
        II = st.tile([PB, CHUNK], F32, name="II")
        G = st.tile([PB, CHUNK], F32, name="G")
        NN = st.tile([PB, CHUNK], F32, name="NN")
        T1 = st.tile([PB, CHUNK], F32, name="T1")
        T2 = st.tile([PB, CHUNK], F32, name="T2")
        XIP = st.tile([PB, CHUNK], F32, name="XIP")
        MSK = st.tile([PB, 1], F32, name="MSK")
        SND = st.tile([PB, 2 * NZ], F32, name="SND")
        B30 = st.tile([PB, 1], F32, name="B30")  # sigmoid bias const
        ACC = st.tile([PB, 2 * len(GROUPS)], F32, name="ACC")  # amr scratch

        def f3(t):  # [PB, FFREE] -> [PB, FYS, FZS]
            return t.rearrange("p (y z) -> p y z", z=FZS)

        def c3(t):  # [PB, CHUNK] -> [PB, SUB, NZ]
            return t.rearrange("p (y z) -> p y z", z=NZ)

        def wg(t, k0, m):  # slots k0..k0+m of W/P -> [PB, m, SUB, NZ]
            return t.rearrange("p (k y z) -> p k y z", k=NOFF, z=NZ)[:, k0 : k0 + m]

        def fint(t):  # own interior of a field tile -> [PB, SUB, NZ]
            return f3(t)[:, 2 : 2 + SUB, 2 : 2 + NZ]

        def xshift(eng, dst, src, dx, rows):
            # dst[p, rows, :] = src[p - dx, rows, :] within each x-block
            # (one DMA per block; DMA has no partition-alignment limits)
            for h in (0, 1):
                base = h * NX
                a, b = base + max(0, dx), base + NX + min(0, dx)
                eng.dma_start(
                    f3(dst)[a:b, rows, :], f3(src)[a - dx : b - dx, rows, :]
                )

        def gwin(base, xs, g):
            # grouped source view: one AP covering every tap in group g,
            # group axis strided by the constant window-offset delta
            dx, taps = g
            t = base if dx == 0 else xs[dx]
            if len(taps) == 1:
                dy, dz = taps[0]
                return f3(t)[:, 2 - dy : 2 - dy + SUB, 2 - dz : 2 - dz + NZ]
            offs = [(2 - dy) * FZS + (2 - dz) for dy, dz in taps]
            delta = offs[1] - offs[0]
            assert delta > 0 and all(
                offs[i + 1] - offs[i] == delta for i in range(len(offs) - 1)
            )
            full = t[:]
            return bass_rust.AP(
                tensor=full.tensor,
                offset=offs[0],
                ap=[[FFREE, PB], [delta, len(taps)], [FZS, SUB], [1, NZ]],
            )

        # ---- init ----
        nc.vector.memset(FA[:], 0.0)
        nc.vector.memset(FB[:], 0.0)
        for dx in XSHIFTS:
            nc.vector.memset(SO[dx][:], 0.0)
        nc.vector.memset(V[:], REST_V)
        nc.vector.memset(B30[:], -0.5 * MIDPOINT)
        nc.sync.dma_start(W[:], w0_d[:])
        nc.sync.dma_start(MSK[:], msk_d[:])

        from concourse.ordered_set import OrderedSet

        _engs = OrderedSet([mybir.EngineType.SP, mybir.EngineType.Activation])
        pid = nc.partition_id(engines=_engs)
        # start row (in the gathered [8*PB, .] buffer) of the left
        # neighbor's top strip / the right neighbor's bottom strip
        selL = nc.snap(
            ((pid + NCORES - 1) % NCORES) * PB + NX,
            engines=_engs, min_val=0, max_val=NCORES * PB - NX,
        )
        selR = nc.snap(
            ((pid + 1) % NCORES) * PB,
            engines=_engs, min_val=0, max_val=NCORES * PB - NX,
        )

        fields = [FA, FB]
        for t in range(nsteps):
            FPREV = fields[t % 2]       # holds out_{t-1} (with halos)
            FOUT = fields[(t + 1) % 2]  # will hold out_t

            if t == 0:
                # external input for step 0 (later steps prefetch at t-1)
                nc.sync.dma_start(XIP[:], xin_d[0])
                nc.scalar.mul(XIP[:], XIP[:], ONE_MINUS_DECAY)

            # ---- syn: grouped shifted products then pairwise tree sum ----
            # products + tree in fp16 (packed 16-bit runs the DVE adds at 2
            # elem/cycle; spike outputs are binary so the ~1e-3 relative
            # rounding only matters within that distance of the threshold)
            k0 = 0
            for g in GROUPS:
                m = len(g[1])
                nc.vector.tensor_tensor(
                    wg(PH, k0, m), wg(W, k0, m), gwin(FPREV, SO, g), ALU.mult
                )
                k0 += m
            # tree-reduce the 24 slots (in place over PH)
            Pf = PH.rearrange("p (k f) -> p k f", k=NOFF)
            nslots = NOFF
            while nslots > 1:
                half = nslots // 2
                nc.vector.tensor_tensor(
                    Pf[:, 0:half],
                    Pf[:, 0:half],
                    Pf[:, half : 2 * half],
                    ALU.add,
                )
                if nslots % 2:
                    nc.vector.tensor_tensor(
                        Pf[:, 0], Pf[:, 0], Pf[:, nslots - 1], ALU.add
                    )
                nslots = half
            # SYN = (P0 * (1-decay)) + XIP ; then v = v*decay + SYN
            nc.vector.scalar_tensor_tensor(
                SYN[:], Pf[:, 0], ONE_MINUS_DECAY, XIP[:], ALU.mult, ALU.add
            )
            nc.vector.scalar_tensor_tensor(
                V[:], V[:], DECAY, SYN[:], ALU.mult, ALU.add
            )

            # ---- neuron update (vector TS + scalar sigmoid) ----
            nc.scalar.activation(G[:], V[:], AT.Sigmoid, bias=B30[:, 0:1], scale=0.5)
            nc.vector.tensor_single_scalar(S[:], V[:], EXC_THR, ALU.is_ge)
            # spike train output for this step (early: S is final here)
            nc.sync.dma_start(spk_d[t], S[:])
            nc.vector.tensor_single_scalar(II[:], V[:], INH_THR, ALU.is_le)
            nc.vector.tensor_tensor(NN[:], S[:], II[:], ALU.add)
            nc.vector.scalar_tensor_tensor(
                T1[:], NN[:], 1.0, G[:], ALU.subtract, ALU.mult
            )
            # out = s - (n-1)*g, written straight into the FOUT interior
            nc.vector.tensor_tensor(fint(FOUT), c3(S), c3(T1), ALU.subtract)

            # ---- intra-core h<->h halo + boundary strips: launch ASAP ----
            # block 0's top halo (ys 6:8) = block 1's own rows 2:4 (p+64);
            # block 1's bottom halo (ys 0:2) = block 0's own rows 4:6 (p-64)
            nc.sync.dma_start(f3(FOUT)[0:NX, 6:8, :], f3(FOUT)[NX:PB, 2:4, :])
            nc.sync.dma_start(f3(FOUT)[NX:PB, 0:2, :], f3(FOUT)[0:NX, 4:6, :])
            # x-shift halo rows whose source is the intra copy (not the
            # network): block 0 row 6, block 1 row 1, for dx=+-1
            for dxp, eng in ((1, nc.scalar), (-1, nc.sync)):
                a, b = max(0, dxp), NX + min(0, dxp)
                eng.dma_start(
                    f3(SO[dxp])[a:b, 6:7, :], f3(FOUT)[a - dxp : b - dxp, 6:7, :]
                )
                eng.dma_start(
                    f3(SO[dxp])[NX + a : NX + b, 1:2, :],
                    f3(FOUT)[NX + a - dxp : NX + b - dxp, 1:2, :],
                )
            # strips: p<64 -> core's bottom 2 rows (ys 2:4 of block 0),
            #         p>=64 -> core's top 2 rows (ys 4:6 of block 1), masked
            snd3 = SND.rearrange("p (y z) -> p y z", z=NZ)
            nc.scalar.activation(
                snd3[0:NX], f3(FOUT)[0:NX, 2:4, 2 : 2 + NZ],
                AT.Copy, bias=0.0, scale=MSK[0:NX, 0:1],
            )
            nc.scalar.activation(
                snd3[NX:PB], f3(FOUT)[NX:PB, 4:6, 2 : 2 + NZ],
                AT.Copy, bias=0.0, scale=MSK[NX:PB, 0:1],
            )
            agin = dr.tile([PB, 2 * NZ], F32, name=f"agin{t}")
            agout = dr.tile(
                [NCORES * PB, 2 * NZ], F32, addr_space="Shared", name=f"agout{t}"
            )
            # posted from scalar, same queue as the masks -> fires immediately
            nc.scalar.dma_start(agin[:], SND[:])
            nc.gpsimd.collective_compute(
                "AllGather",
                ALU.bypass,
                replica_groups=[list(range(NCORES))],
                ins=[agin.opt()],
                outs=[agout.opt()],
            )

            # ---- off the critical path while the collective flies ----
            # urgent x-shifted copies of out_t: interior rows 2:6 only (all
            # that part A and the next step's dy==0 syn taps read), posted
            # from both DMA-capable queues in parallel
            xshift(nc.scalar, SO[1], FOUT, 1, slice(2, 6))
            xshift(nc.scalar, SO[-1], FOUT, -1, slice(2, 6))
            xshift(nc.sync, SO[2], FOUT, 2, slice(2, 6))
            xshift(nc.sync, SO[-2], FOUT, -2, slice(2, 6))
            # prefetch + prescale next step's external input (WAR on this
            # step's SYN read is tracked automatically)
            if t + 1 < nsteps:
                nc.sync.dma_start(XIP[:], xin_d[t + 1])
                nc.scalar.mul(XIP[:], XIP[:], ONE_MINUS_DECAY)
            # v reset where spiked: v -= (v - RESET_V) * s
            nc.vector.scalar_tensor_tensor(
                T2[:], V[:], -RESET_V, S[:], ALU.add, ALU.mult
            )
            nc.vector.tensor_tensor(V[:], V[:], T2[:], ALU.subtract)

            # ---- STDP part A: dy == 0 groups need no y-halo (t=0 skips) ----
            # dw[k][j] = prev[j] * (0.015*out[j-dk] - 0.005) via fused DVE op
            if t > 0:
                for k, d in enumerate(OFFSETS[:NOFF_A]):
                    dxk, dy, dz = d
                    src = FOUT if dxk == 0 else SO[dxk]
                    nc.vector.affine_mul_reduce(
                        wg(P, k, 1).squeeze(1),
                        ACC[:, k : k + 1],
                        f3(src)[:, 2 - dy : 2 - dy + SUB, 2 - dz : 2 - dz + NZ],
                        fint(FPREV),
                        ETA_LTP + ETA_LTD,
                        -ETA_LTD,
                    )
                nc.vector.scalar_tensor_tensor(
                    W[:, 0 : NOFF_A * CHUNK],
                    W[:, 0 : NOFF_A * CHUNK],
                    1.0 - WDECAY,
                    P[:, 0 : NOFF_A * CHUNK],
                    ALU.mult,
                    ALU.add,
                )
                nc.vector.tensor_scalar(
                    W[:, 0 : NOFF_A * CHUNK],
                    W[:, 0 : NOFF_A * CHUNK],
                    1.0,
                    0.0,
                    ALU.min,
                    ALU.max,
                )

            # ---- halo extraction (gated on the collective; posts spread
            # across sync+scalar queues so they all fire in parallel) ----
            agf = agout.rearrange("p (y z) -> p y z", z=NZ)
            # left neighbor's top strip -> block 0's bottom halo (ys 0:2)
            nc.sync.dma_start(
                f3(FOUT)[0:NX, 0:2, 2 : 2 + NZ],
                agf[bass.ds(selL, NX)],
            )
            # right neighbor's bottom strip -> block 1's top halo (ys 6:8)
            nc.scalar.dma_start(
                f3(FOUT)[NX:PB, 6:8, 2 : 2 + NZ],
                agf[bass.ds(selR, NX)],
            )
            # x-shifted copies of the exchanged halo rows, read straight out
            # of the gathered buffer with a partition offset (no extract ->
            # patch chain): SO[dx] block 0 row 1 and block 1 row 6
            for dxp, engs in ((1, (nc.sync, nc.scalar)), (-1, (nc.scalar, nc.sync))):
                a, b = max(0, dxp), NX + min(0, dxp)
                engs[0].dma_start(
                    f3(SO[dxp])[a:b, 1:2, 2 : 2 + NZ],
                    agf[bass.ds(selL + a - dxp, b - a), 1:2],
                )
                engs[1].dma_start(
                    f3(SO[dxp])[NX + a : NX + b, 6:7, 2 : 2 + NZ],
                    agf[bass.ds(selR + a - dxp, b - a), 0:1],
                )

            # ---- STDP part B: dy != 0 groups (need the fresh halo) ----
            if t > 0:
                for k, d in enumerate(OFFSETS[NOFF_A:], start=NOFF_A):
                    dxk, dy, dz = d
                    src = FOUT if dxk == 0 else SO[dxk]
                    nc.vector.affine_mul_reduce(
                        wg(P, k, 1).squeeze(1),
                        ACC[:, k : k + 1],
                        f3(src)[:, 2 - dy : 2 - dy + SUB, 2 - dz : 2 - dz + NZ],
                        fint(FPREV),
                        ETA_LTP + ETA_LTD,
                        -ETA_LTD,
                    )
                nc.vector.scalar_tensor_tensor(
                    W[:, NOFF_A * CHUNK :],
                    W[:, NOFF_A * CHUNK :],
                    1.0 - WDECAY,
                    P[:, NOFF_A * CHUNK :],
                    ALU.mult,
                    ALU.add,
                )
                nc.vector.tensor_scalar(
                    W[:, NOFF_A * CHUNK :],
                    W[:, NOFF_A * CHUNK :],
                    1.0,
                    0.0,
                    ALU.min,
                    ALU.max,
                )

    nc.compile()
    return nc


def _shard_inputs(external_input, edge_values, edge_rows, edge_cols, nsteps):
    """Build per-core input maps (host-side sharding)."""
    ext = np.ascontiguousarray(np.asarray(external_input, dtype=np.float32))[:nsteps]
    vals = np.asarray(edge_values, dtype=np.float32)
    rows = np.asarray(edge_rows, dtype=np.int64)
    cols = np.asarray(edge_cols, dtype=np.int64)

    # dense weights keyed by destination: Wd[k, j] = w(edge j-d_k -> j)
    dlin = cols - rows
    offs_lin = np.array([d[0] * NY * NZ + d[1] * NZ + d[2] for d in OFFSETS])
    assert set(int(v) for v in np.unique(dlin)).issubset(
        set(int(v) for v in offs_lin)
    )
    k_of = np.zeros(int(offs_lin.max()) - int(offs_lin.min()) + 1, dtype=np.int64)
    for i, v in enumerate(offs_lin):
        k_of[int(v) - int(offs_lin.min())] = i
    ke = k_of[dlin - int(offs_lin.min())]
    Wd = np.zeros((NOFF, N), dtype=np.float32)
    Wd[ke, cols] = vals

    # [NOFF, NX, NCORES, 2(h), SUB, NZ]
    Wd = Wd.reshape(NOFF, NX, NCORES, 2, SUB, NZ)
    ext = ext.reshape(nsteps, NX, NCORES, 2, SUB, NZ)

    in_maps = []
    for c in range(NCORES):
        # partition p = h*64 + x
        wc = np.ascontiguousarray(
            Wd[:, :, c].transpose(2, 1, 0, 3, 4)
        ).reshape(PB, NOFF * CHUNK)
        xc = np.ascontiguousarray(
            ext[:, :, c].transpose(0, 2, 1, 3, 4)
        ).reshape(nsteps, PB, CHUNK)
        msk = np.zeros((PB, 1), dtype=np.float32)
        msk[0:NX, 0] = 0.0 if c == 0 else 1.0           # bottom strip valid?
        msk[NX:PB, 0] = 0.0 if c == NCORES - 1 else 1.0  # top strip valid?
        in_maps.append({"w0": wc, "xin": xc, "msk": msk})
    return in_maps


def kernel(external_input, edge_values, edge_rows, edge_cols, num_steps):
    nsteps = int(num_steps)
    if nsteps not in _CACHE:
        _CACHE[nsteps] = _build_graph(nsteps)
    nc = _CACHE[nsteps]

    in_maps = _shard_inputs(external_input, edge_values, edge_rows, edge_cols, nsteps)
    res = bass_utils.run_bass_kernel_spmd(
        nc,
        in_maps,
        core_ids=list(range(NCORES)),
        trace=bool(int(os.environ.get("BRAIN_TRACE", "0"))),
    )

    out = np.empty((nsteps, NX, NCORES, 2, SUB, NZ), dtype=np.float32)
    for c in range(NCORES):
        out[:, :, c] = (
            res.results[c]["spk"]
            .reshape(nsteps, 2, NX, SUB, NZ)
            .transpose(0, 2, 1, 3, 4)
        )
    kernel.last_results = res
    return out.reshape(nsteps, N)


# revision 30
# speedup vs baseline: 1.1582x; 1.0784x over previous
"""Trainium2 Bass kernel for Brain3DQTUNNetwork (gnn_message_passing).

The "sparse" graph is a fixed Manhattan-radius-2 stencil on a 64^3 grid
(24 offsets).  Weights are stored dense per offset slot, keyed by the
DESTINATION (col) index: W[k][j] = w(edge j-d_k -> j), 0 for invalid
edges.  The per-step segment_sum SpMV becomes 24 shifted elementwise
multiply-accumulates, and the STDP update becomes
    w = clip(w*(1-WDECAY) + prev * q_shift,  0, 1),   q(o) = 0.015*o - 0.005.
Invalid slots self-heal to 0 every step (q reads 0 / q(0) < 0 there and
the clip floors at 0).

Layout: 128 partitions = (y-half h, x): p = h*64 + x.  Each partition
holds a 4-y-plane sub-slab: field free axis = (y_sub + halo, z + pad) =
8*68 = 544.  y/z shifts are free-dim AP offsets; x shifts are 2
per-block SBUF->SBUF DMA partition-shifted copies; the h<->h halo is an
intra-core partition+-64 DMA copy.

Sharding: 8 cores x 8 y-planes.  All state stays SBUF-resident; per-step
cross-core traffic is one 8-rank AllGather of 2-row boundary strips.

Engine use: everything elementwise on Vector (DVE); sigmoid + masked
strip copies on Scalar; GpSimd only fires the collective.  The STDP
product uses the fused affine_mul_reduce DVE op (folds q() into the
multiply).  Offset slots are grouped so stencil taps whose windows
differ by a constant stride share one instruction (24 taps -> 12 ops),
with dy==0 groups first so half the STDP/W update overlaps the
AllGather latency.
"""

import os
import sys

sys.path.insert(0, "/opt/trn_rl_repo")

import numpy as np

import bass_rust
import concourse.bass as bass
import concourse.bacc as bacc
import concourse.mybir as mybir
import concourse.tile as tile
from concourse import bass_utils

# ---- problem constants (hardcoded; kernel.py must be self-contained) ----
GRID = (64, 64, 64)
NX, NY, NZ = GRID
N = NX * NY * NZ
RADIUS = 2
NCORES = 8
YS = NY // NCORES  # y-planes per core = 8

TAU = 20.0
REST_V = -65.0
EXC_THR = -50.0
INH_THR = -70.0
RESET_V = -65.0
ETA_LTP, ETA_LTD, WDECAY = 0.01, 0.005, 1e-05

# fp32-exact scalars matching the jax reference
DECAY = float(np.exp(np.float32(-1.0 / np.float32(TAU))).astype(np.float32))
ONE_MINUS_DECAY = float(np.float32(1.0) - np.float32(DECAY))
MIDPOINT = (EXC_THR + INH_THR) / 2.0  # -60.0

# ---- offset slot grouping ----
# Each group = (dx, [(dy, dz), ...]) where consecutive taps' field windows
# differ by a constant positive stride, so one strided AP covers the whole
# group.  Part A (dy == 0, k 0..11) needs no y-halo; part B (k 12..23) does.
GROUPS_A = [
    (0, [(0, 2), (0, 1)]),            # window offsets 136,137 (stride 1)
    (0, [(0, -1), (0, -2)]),          # 139,140
    (1, [(0, 1), (0, 0), (0, -1)]),   # 137,138,139
    (-1, [(0, 1), (0, 0), (0, -1)]),
    (2, [(0, 0)]),
    (-2, [(0, 0)]),
]
GROUPS_B = [
    (0, [(2, 0)]),                    # 2
    (0, [(1, 1), (1, 0), (1, -1)]),   # 69,70,71
    (0, [(-1, 1), (-1, 0), (-1, -1)]),  # 205,206,207
    (0, [(-2, 0)]),                   # 274
    (1, [(1, 0), (-1, 0)]),           # 70,206 (stride 136)
    (-1, [(1, 0), (-1, 0)]),
]
GROUPS = GROUPS_A + GROUPS_B
OFFSETS = [(dx, dy, dz) for dx, tap in GROUPS for dy, dz in tap]
NOFF = len(OFFSETS)  # 24
NOFF_A = sum(len(tap) for _, tap in GROUPS_A)  # 12
XSHIFTS = (1, -1, 2, -2)

# field geometry: partition p = h*64 + x, free = ys*68 + zs,
# ys = y_sub + 2 in [0,8), zs = z + 2 in [0,68)
PB = 2 * NX        # 128 partitions
SUB = YS // 2      # 4 own y rows per partition block
FYS = SUB + 4      # 8 field y rows (4 own + 2 halo each side)
FZS = NZ + 4       # 68 field z cols
FFREE = FYS * FZS  # 544
CHUNK = SUB * NZ   # 256 elems per partition for compact tiles
F32 = mybir.dt.float32
F16 = mybir.dt.float16

_CACHE = {}


def _build_graph(nsteps):
    nc = bacc.Bacc(
        "TRN2",
        target_bir_lowering=False,
        debug=False,
        enable_asserts=True,
        num_devices=NCORES,
    )
    w0_d = nc.dram_tensor("w0", [PB, NOFF * CHUNK], F32, kind="ExternalInput").ap()
    xin_d = nc.dram_tensor("xin", [nsteps, PB, CHUNK], F32, kind="ExternalInput").ap()
    msk_d = nc.dram_tensor("msk", [PB, 1], F32, kind="ExternalInput").ap()
    spk_d = nc.dram_tensor("spk", [nsteps, PB, CHUNK], F32, kind="ExternalOutput").ap()

    AT = mybir.ActivationFunctionType
    ALU = mybir.AluOpType

    with tile.TileContext(nc) as tc, tc.tile_pool(
        name="state", bufs=1
    ) as st, tc.tile_pool(name="dram", bufs=1, space="DRAM") as dr:
        # persistent state tiles (distinct names -> distinct slots)
        W = st.tile([PB, NOFF * CHUNK], F32, name="W")
        P = st.tile([PB, NOFF * CHUNK], F32, name="P")
        PH = st.tile([PB, NOFF * CHUNK], F16, name="PH")
        WH = st.tile([PB, NOFF * CHUNK], F16, name="WH")  # fp16 shadow of W
        FA = st.tile([PB, FFREE], F16, name="FA")
        FB = st.tile([PB, FFREE], F16, name="FB")
        SO = {dx: st.tile([PB, FFREE], F16, name=f"SOx{dx+2}") for dx in XSHIFTS}
        V = st.tile([PB, CHUNK], F32, name="V")
        SYN = st.tile([PB, CHUNK], F32, name="SYN")
        S = st.tile([PB, CHUNK], F32, name="S")
        II = st.tile([PB, CHUNK], F32, name="II")
        G = st.tile([PB, CHUNK], F32, name="G")
        NN = st.tile([PB, CHUNK], F32, name="NN")
        T1 = st.tile([PB, CHUNK], F32, name="T1")
        T2 = st.tile([PB, CHUNK], F32, name="T2")
        XIP = st.tile([PB, CHUNK], F32, name="XIP")
        MSK = st.tile([PB, 1], F32, name="MSK")
        SND = st.tile([PB, 2 * NZ], F16, name="SND")
        B30 = st.tile([PB, 1], F32, name="B30")  # sigmoid bias const
        ACC = st.tile([PB, 2 * len(GROUPS)], F32, name="ACC")  # amr scratch

        def f3(t):  # [PB, FFREE] -> [PB, FYS, FZS]
            return t.rearrange("p (y z) -> p y z", z=FZS)

        def c3(t):  # [PB, CHUNK] -> [PB, SUB, NZ]
            return t.rearrange("p (y z) -> p y z", z=NZ)

        def wg(t, k0, m):  # slots k0..k0+m of W/P -> [PB, m, SUB, NZ]
            return t.rearrange("p (k y z) -> p k y z", k=NOFF, z=NZ)[:, k0 : k0 + m]

        def fint(t):  # own interior of a field tile -> [PB, SUB, NZ]
            return f3(t)[:, 2 : 2 + SUB, 2 : 2 + NZ]

        def xshift(eng, dst, src, dx, rows):
            # dst[p, rows, :] = src[p - dx, rows, :] within each x-block
            # (one DMA per block; DMA has no partition-alignment limits)
            for h in (0, 1):
                base = h * NX
                a, b = base + max(0, dx), base + NX + min(0, dx)
                eng.dma_start(
                    f3(dst)[a:b, rows, :], f3(src)[a - dx : b - dx, rows, :]
                )

        def gwin(base, xs, g):
            # grouped source view: one AP covering every tap in group g,
            # group axis strided by the constant window-offset delta
            dx, taps = g
            t = base if dx == 0 else xs[dx]
            if len(taps) == 1:
                dy, dz = taps[0]
                return f3(t)[:, 2 - dy : 2 - dy + SUB, 2 - dz : 2 - dz + NZ]
            offs = [(2 - dy) * FZS + (2 - dz) for dy, dz in taps]
            delta = offs[1] - offs[0]
            assert delta > 0 and all(
                offs[i + 1] - offs[i] == delta for i in range(len(offs) - 1)
            )
            full = t[:]
            return bass_rust.AP(
                tensor=full.tensor,
                offset=offs[0],
                ap=[[FFREE, PB], [delta, len(taps)], [FZS, SUB], [1, NZ]],
            )

        # ---- init ----
        nc.vector.memset(FA[:], 0.0)
        nc.vector.memset(FB[:], 0.0)
        for dx in XSHIFTS:
            nc.vector.memset(SO[dx][:], 0.0)
        nc.vector.memset(V[:], REST_V)
        nc.vector.memset(B30[:], -0.5 * MIDPOINT)
        nc.sync.dma_start(W[:], w0_d[:])
        nc.vector.tensor_scalar(WH[:], W[:], 1.0, 0.0, ALU.min, ALU.max)
        nc.sync.dma_start(MSK[:], msk_d[:])

        from concourse.ordered_set import OrderedSet

        _engs = OrderedSet([mybir.EngineType.SP, mybir.EngineType.Activation])
        pid = nc.partition_id(engines=_engs)
        # start row (in the gathered [8*PB, .] buffer) of the left
        # neighbor's top strip / the right neighbor's bottom strip
        selL = nc.snap(
            ((pid + NCORES - 1) % NCORES) * PB + NX,
            engines=_engs, min_val=0, max_val=NCORES * PB - NX,
        )
        selR = nc.snap(
            ((pid + 1) % NCORES) * PB,
            engines=_engs, min_val=0, max_val=NCORES * PB - NX,
        )

        fields = [FA, FB]
        for t in range(nsteps):
            FPREV = fields[t % 2]       # holds out_{t-1} (with halos)
            FOUT = fields[(t + 1) % 2]  # will hold out_t

            if t == 0:
                # external input for step 0 (later steps prefetch at t-1)
                nc.sync.dma_start(XIP[:], xin_d[0])
                nc.scalar.mul(XIP[:], XIP[:], ONE_MINUS_DECAY)

            # ---- syn: grouped shifted products then pairwise tree sum ----
            # products + tree in fp16 (packed 16-bit runs the DVE adds at 2
            # elem/cycle; spike outputs are binary so the ~1e-3 relative
            # rounding only matters within that distance of the threshold)
            k0 = 0
            for g in GROUPS:
                m = len(g[1])
                nc.vector.tensor_tensor(
                    wg(PH, k0, m), wg(WH, k0, m), gwin(FPREV, SO, g), ALU.mult
                )
                k0 += m
            # tree-reduce the 24 slots (in place over PH)
            Pf = PH.rearrange("p (k f) -> p k f", k=NOFF)
            nslots = NOFF
            while nslots > 1:
                half = nslots // 2
                nc.vector.tensor_tensor(
                    Pf[:, 0:half],
                    Pf[:, 0:half],
                    Pf[:, half : 2 * half],
                    ALU.add,
                )
                if nslots % 2:
                    nc.vector.tensor_tensor(
                        Pf[:, 0], Pf[:, 0], Pf[:, nslots - 1], ALU.add
                    )
                nslots = half
            # SYN = (P0 * (1-decay)) + XIP ; then v = v*decay + SYN
            nc.vector.scalar_tensor_tensor(
                SYN[:], Pf[:, 0], ONE_MINUS_DECAY, XIP[:], ALU.mult, ALU.add
            )
            nc.vector.scalar_tensor_tensor(
                V[:], V[:], DECAY, SYN[:], ALU.mult, ALU.add
            )

            # ---- neuron update (vector TS + scalar sigmoid) ----
            nc.scalar.activation(G[:], V[:], AT.Sigmoid, bias=B30[:, 0:1], scale=0.5)
            nc.vector.tensor_single_scalar(S[:], V[:], EXC_THR, ALU.is_ge)
            # spike train output for this step (early: S is final here)
            nc.sync.dma_start(spk_d[t], S[:])
            nc.vector.tensor_single_scalar(II[:], V[:], INH_THR, ALU.is_le)
            nc.vector.tensor_tensor(NN[:], S[:], II[:], ALU.add)
            nc.vector.scalar_tensor_tensor(
                T1[:], NN[:], 1.0, G[:], ALU.subtract, ALU.mult
            )
            # out = s - (n-1)*g, written straight into the FOUT interior
            nc.vector.tensor_tensor(fint(FOUT), c3(S), c3(T1), ALU.subtract)

            # ---- intra-core h<->h halo + boundary strips: launch ASAP ----
            # block 0's top halo (ys 6:8) = block 1's own rows 2:4 (p+64);
            # block 1's bottom halo (ys 0:2) = block 0's own rows 4:6 (p-64)
            nc.sync.dma_start(f3(FOUT)[0:NX, 6:8, :], f3(FOUT)[NX:PB, 2:4, :])
            nc.sync.dma_start(f3(FOUT)[NX:PB, 0:2, :], f3(FOUT)[0:NX, 4:6, :])
            # x-shift halo rows whose source is the intra copy (not the
            # network): block 0 row 6, block 1 row 1, for dx=+-1
            for dxp, eng in ((1, nc.scalar), (-1, nc.sync)):
                a, b = max(0, dxp), NX + min(0, dxp)
                eng.dma_start(
                    f3(SO[dxp])[a:b, 6:7, :], f3(FOUT)[a - dxp : b - dxp, 6:7, :]
                )
                eng.dma_start(
                    f3(SO[dxp])[NX + a : NX + b, 1:2, :],
                    f3(FOUT)[NX + a - dxp : NX + b - dxp, 1:2, :],
                )
            # strips: p<64 -> core's bottom 2 rows (ys 2:4 of block 0),
            #         p>=64 -> core's top 2 rows (ys 4:6 of block 1), masked
            snd3 = SND.rearrange("p (y z) -> p y z", z=NZ)
            nc.scalar.activation(
                snd3[0:NX], f3(FOUT)[0:NX, 2:4, 2 : 2 + NZ],
                AT.Copy, bias=0.0, scale=MSK[0:NX, 0:1],
            )
            nc.scalar.activation(
                snd3[NX:PB], f3(FOUT)[NX:PB, 4:6, 2 : 2 + NZ],
                AT.Copy, bias=0.0, scale=MSK[NX:PB, 0:1],
            )
            agin = dr.tile([PB, 2 * NZ], F16, name=f"agin{t}")
            agout = dr.tile(
                [NCORES * PB, 2 * NZ], F16, addr_space="Shared", name=f"agout{t}"
            )
            # posted from scalar, same queue as the masks -> fires immediately
            nc.scalar.dma_start(agin[:], SND[:])
            nc.gpsimd.collective_compute(
                "AllGather",
                ALU.bypass,
                replica_groups=[list(range(NCORES))],
                ins=[agin.opt()],
                outs=[agout.opt()],
            )

            # ---- off the critical path while the collective flies ----
            # urgent x-shifted copies of out_t: interior rows 2:6 only (all
            # that part A and the next step's dy==0 syn taps read), posted
            # from both DMA-capable queues in parallel
            xshift(nc.scalar, SO[1], FOUT, 1, slice(2, 6))
            xshift(nc.scalar, SO[-1], FOUT, -1, slice(2, 6))
            xshift(nc.sync, SO[2], FOUT, 2, slice(2, 6))
            xshift(nc.sync, SO[-2], FOUT, -2, slice(2, 6))
            # prefetch + prescale next step's external input (WAR on this
            # step's SYN read is tracked automatically)
            if t + 1 < nsteps:
                nc.sync.dma_start(XIP[:], xin_d[t + 1])
                nc.scalar.mul(XIP[:], XIP[:], ONE_MINUS_DECAY)
            # v reset where spiked: v -= (v - RESET_V) * s
            nc.vector.scalar_tensor_tensor(
                T2[:], V[:], -RESET_V, S[:], ALU.add, ALU.mult
            )
            nc.vector.tensor_tensor(V[:], V[:], T2[:], ALU.subtract)

            # ---- STDP part A: dy == 0 groups need no y-halo (t=0 skips) ----
            # dw[k][j] = prev[j] * (0.015*out[j-dk] - 0.005) via fused DVE op
            if t > 0:
                for k, d in enumerate(OFFSETS[:NOFF_A]):
                    dxk, dy, dz = d
                    src = FOUT if dxk == 0 else SO[dxk]
                    nc.vector.affine_mul_reduce(
                        wg(P, k, 1).squeeze(1),
                        ACC[:, k : k + 1],
                        f3(src)[:, 2 - dy : 2 - dy + SUB, 2 - dz : 2 - dz + NZ],
                        fint(FPREV),
                        ETA_LTP + ETA_LTD,
                        -ETA_LTD,
                    )
                nc.vector.scalar_tensor_tensor(
                    W[:, 0 : NOFF_A * CHUNK],
                    W[:, 0 : NOFF_A * CHUNK],
                    1.0 - WDECAY,
                    P[:, 0 : NOFF_A * CHUNK],
                    ALU.mult,
                    ALU.add,
                )
                nc.vector.tensor_scalar(
                    W[:, 0 : NOFF_A * CHUNK],
                    W[:, 0 : NOFF_A * CHUNK],
                    1.0,
                    0.0,
                    ALU.min,
                    ALU.max,
                )
                nc.vector.tensor_scalar(
                    WH[:, 0 : NOFF_A * CHUNK],
                    W[:, 0 : NOFF_A * CHUNK],
                    1.0,
                    0.0,
                    ALU.min,
                    ALU.max,
                )

            # ---- halo extraction (gated on the collective; posts spread
            # across sync+scalar queues so they all fire in parallel) ----
            agf = agout.rearrange("p (y z) -> p y z", z=NZ)
            # left neighbor's top strip -> block 0's bottom halo (ys 0:2)
            nc.sync.dma_start(
                f3(FOUT)[0:NX, 0:2, 2 : 2 + NZ],
                agf[bass.ds(selL, NX)],
            )
            # right neighbor's bottom strip -> block 1's top halo (ys 6:8)
            nc.scalar.dma_start(
                f3(FOUT)[NX:PB, 6:8, 2 : 2 + NZ],
                agf[bass.ds(selR, NX)],
            )
            # x-shifted copies of the exchanged halo rows, read straight out
            # of the gathered buffer with a partition offset (no extract ->
            # patch chain): SO[dx] block 0 row 1 and block 1 row 6
            for dxp, engs in ((1, (nc.sync, nc.scalar)), (-1, (nc.scalar, nc.sync))):
                a, b = max(0, dxp), NX + min(0, dxp)
                engs[0].dma_start(
                    f3(SO[dxp])[a:b, 1:2, 2 : 2 + NZ],
                    agf[bass.ds(selL + a - dxp, b - a), 1:2],
                )
                engs[1].dma_start(
                    f3(SO[dxp])[NX + a : NX + b, 6:7, 2 : 2 + NZ],
                    agf[bass.ds(selR + a - dxp, b - a), 0:1],
                )

            # ---- STDP part B: dy != 0 groups (need the fresh halo) ----
            if t > 0:
                for k, d in enumerate(OFFSETS[NOFF_A:], start=NOFF_A):
                    dxk, dy, dz = d
                    src = FOUT if dxk == 0 else SO[dxk]
                    nc.vector.affine_mul_reduce(
                        wg(P, k, 1).squeeze(1),
                        ACC[:, k : k + 1],
                        f3(src)[:, 2 - dy : 2 - dy + SUB, 2 - dz : 2 - dz + NZ],
                        fint(FPREV),
                        ETA_LTP + ETA_LTD,
                        -ETA_LTD,
                    )
                nc.vector.scalar_tensor_tensor(
                    W[:, NOFF_A * CHUNK :],
                    W[:, NOFF_A * CHUNK :],
                    1.0 - WDECAY,
                    P[:, NOFF_A * CHUNK :],
                    ALU.mult,
                    ALU.add,
                )
                nc.vector.tensor_scalar(
                    W[:, NOFF_A * CHUNK :],
                    W[:, NOFF_A * CHUNK :],
                    1.0,
                    0.0,
                    ALU.min,
                    ALU.max,
                )
                nc.vector.tensor_scalar(
                    WH[:, NOFF_A * CHUNK :],
                    W[:, NOFF_A * CHUNK :],
                    1.0,
                    0.0,
                    ALU.min,
                    ALU.max,
                )

    nc.compile()
    return nc


def _shard_inputs(external_input, edge_values, edge_rows, edge_cols, nsteps):
    """Build per-core input maps (host-side sharding)."""
    ext = np.ascontiguousarray(np.asarray(external_input, dtype=np.float32))[:nsteps]
    vals = np.asarray(edge_values, dtype=np.float32)
    rows = np.asarray(edge_rows, dtype=np.int64)
    cols = np.asarray(edge_cols, dtype=np.int64)

    # dense weights keyed by destination: Wd[k, j] = w(edge j-d_k -> j)
    dlin = cols - rows
    offs_lin = np.array([d[0] * NY * NZ + d[1] * NZ + d[2] for d in OFFSETS])
    assert set(int(v) for v in np.unique(dlin)).issubset(
        set(int(v) for v in offs_lin)
    )
    k_of = np.zeros(int(offs_lin.max()) - int(offs_lin.min()) + 1, dtype=np.int64)
    for i, v in enumerate(offs_lin):
        k_of[int(v) - int(offs_lin.min())] = i
    ke = k_of[dlin - int(offs_lin.min())]
    Wd = np.zeros((NOFF, N), dtype=np.float32)
    Wd[ke, cols] = vals

    # [NOFF, NX, NCORES, 2(h), SUB, NZ]
    Wd = Wd.reshape(NOFF, NX, NCORES, 2, SUB, NZ)
    ext = ext.reshape(nsteps, NX, NCORES, 2, SUB, NZ)

    in_maps = []
    for c in range(NCORES):
        # partition p = h*64 + x
        wc = np.ascontiguousarray(
            Wd[:, :, c].transpose(2, 1, 0, 3, 4)
        ).reshape(PB, NOFF * CHUNK)
        xc = np.ascontiguousarray(
            ext[:, :, c].transpose(0, 2, 1, 3, 4)
        ).reshape(nsteps, PB, CHUNK)
        msk = np.zeros((PB, 1), dtype=np.float32)
        msk[0:NX, 0] = 0.0 if c == 0 else 1.0           # bottom strip valid?
        msk[NX:PB, 0] = 0.0 if c == NCORES - 1 else 1.0  # top strip valid?
        in_maps.append({"w0": wc, "xin": xc, "msk": msk})
    return in_maps


def kernel(external_input, edge_values, edge_rows, edge_cols, num_steps):
    nsteps = int(num_steps)
    if nsteps not in _CACHE:
        _CACHE[nsteps] = _build_graph(nsteps)
    nc = _CACHE[nsteps]

    in_maps = _shard_inputs(external_input, edge_values, edge_rows, edge_cols, nsteps)
    res = bass_utils.run_bass_kernel_spmd(
        nc,
        in_maps,
        core_ids=list(range(NCORES)),
        trace=bool(int(os.environ.get("BRAIN_TRACE", "0"))),
    )

    out = np.empty((nsteps, NX, NCORES, 2, SUB, NZ), dtype=np.float32)
    for c in range(NCORES):
        out[:, :, c] = (
            res.results[c]["spk"]
            .reshape(nsteps, 2, NX, SUB, NZ)
            .transpose(0, 2, 1, 3, 4)
        )
    kernel.last_results = res
    return out.reshape(nsteps, N)
